# revision 37
# baseline (speedup 1.0000x reference)
"""Trainium2 Bass kernel for nn_CPA_43 (dense transformer block, CPA attention).

Data-parallel over batch: B=256 sharded as 32 samples per core across 8 cores.
All weights replicated. Two on-chip stages per core:
  stage 1: LN1/LN2, Q/K/V projections, channel-softmax(q), position-softmax(k),
           context/attention matmuls, Wr + residual -> f3out (spilled to DRAM),
           LN3 statistics on the fly (bn_stats).
  stage 2: LN3 apply, MLP (W1 -> gelu -> W2) + residual -> output.

Projection/MLP matmuls run in float32r (full PE rate at free dim >= 256,
~1.5e-4 rel err). The attention inner path (softmaxed q/k, v, context) runs in
bf16 — full PE rate at any free dim. Per-channel / per-position biases are
preloaded into PSUM with identity/ones matmuls; LN gains and the positional
projections are folded into weights/biases on the host.

Activation-table sets are pinned to natural_log_exp_and_others (stage 1) and
gelu_and_others (stage 2) to avoid ~2.7us table reloads.
"""

import numpy as np

B, N3, N4, DIM, HEADS, MLP_DIM = 256, 256, 64, 512, 8, 2048
N_CORES = 8
BSH = B // N_CORES  # samples per core
EPS = 1e-5
SG4 = 8  # f4-group size (samples per K/V block)
WSCALE = 2.0 ** 10   # fp8 weight scale (weights ~N(0,.02) would underflow fp8)
DESCALE = 1.0 / WSCALE

_BUILD_CACHE = {}


def _host_prep(inputs):
    """Fold LN gains + positional projections into weights/biases (exact)."""
    f = {k: np.asarray(v, dtype=np.float64) for k, v in inputs.items()}
    pos3 = f["pos3"][0]  # [N3, DIM]
    pos4 = f["pos4"][0]  # [N4, DIM]

    import ml_dtypes
    fp8 = ml_dtypes.float8_e4m3

    def q8(w):  # weights are ~N(0, 0.02): scale 2^10 into fp8's sweet spot
        return np.ascontiguousarray(np.clip(w * WSCALE, -240, 240).astype(fp8))

    bf16 = ml_dtypes.bfloat16
    wq = np.ascontiguousarray((f["ln1_g"][:, None] * f["Wq"]).astype(bf16))
    wk = np.ascontiguousarray((f["ln2_g"][:, None] * f["Wk"]).astype(bf16))
    wv = np.ascontiguousarray((f["ln2_g"][:, None] * f["Wv"]).astype(bf16))
    wr = q8(f["Wr"])
    w1 = q8(f["ln3_g"][:, None] * f["W1"])
    w2 = q8(f["W2"])

    biasq = ((f["ln1_b"][None, :] + pos3) @ f["Wq"] + f["bq"]).astype(bf16)
    biask = ((f["ln2_b"][None, :] + pos4) @ f["Wk"] + f["bk"]).astype(bf16)
    bias1 = (f["ln3_b"] @ f["W1"] + f["b1"]).astype(np.float32)  # [MLP] (post-descale)
    # The V bias passes through attention unchanged (softmax(k) sums to 1 over
    # positions, softmax(q) sums to 1 over head channels), so fold it into br.
    biasv = f["ln2_b"] @ f["Wv"] + f["bv"]  # [DIM]
    br = ((f["br"] + biasv @ f["Wr"]) * WSCALE).astype(np.float32)
    b2 = (f["b2"] * WSCALE).astype(np.float32)

    # biask in channel-major, tiled over the SG4 samples of an f4-group:
    # [DIM, SG4*N4] with column order (sample_in_group, position)
    biask_cm = np.tile(biask.T[:, None, :], (1, SG4, 1)).reshape(DIM, SG4 * N4)
    biask_cm = np.ascontiguousarray(biask_cm.astype(bf16))
    # bias1 as [128, 16]: column hc holds biases for hidden channels hc*128..+128
    bias1_cm = np.ascontiguousarray(bias1.reshape(MLP_DIM // 128, 128).T.astype(np.float32))

    return {
        "wq": wq, "wk": wk, "wv": wv, "wr": wr, "w1": w1, "w2": w2,
        "biasq": np.ascontiguousarray(biasq),
        "biask_cm": biask_cm,
        "br_row": np.ascontiguousarray(br[None, :]),
        "b2_row": np.ascontiguousarray(b2[None, :]),
        "bias1_cm": bias1_cm,
        "ones_col": np.ones((1, 128), dtype=np.float32),
        "ident": np.eye(128, dtype=np.float32),
        "ident_bf": np.eye(128, dtype=np.float32),  # cast to bf16 tile on chip
    }


def _build(n_samples, repeat=1):
    """Build the Bacc module for one core processing `n_samples` samples.

    `repeat` re-runs the whole computation that many times back-to-back —
    used only for wall-clock timing amplification in test.py."""
    import concourse.bacc as bacc
    import concourse.tile as tile
    import concourse.mybir as mybir
    from concourse.bass import AP  # noqa: F401

    # Restrict activation-table-set choices (see module docstring).
    if not hasattr(bacc, "_orig_get_activation_tables"):
        bacc._orig_get_activation_tables = bacc.get_activation_tables

        def _gat(arch):
            full = bacc._orig_get_activation_tables(arch)
            keep = {"natural_log_exp_and_others", "gelu_and_others"}
            return {n: (s if n in keep else set()) for n, s in full.items()}

        bacc.get_activation_tables = _gat

    F32 = mybir.dt.float32
    F32R = mybir.dt.float32r
    BF16 = mybir.dt.bfloat16
    FP8 = mybir.dt.float8e4
    DR = mybir.MatmulPerfMode.DoubleRow
    AX = mybir.AxisListType.X
    ALU = mybir.AluOpType
    ACTF = mybir.ActivationFunctionType

    NS = n_samples
    assert NS % SG4 == 0
    NG4 = NS // SG4      # f4 groups
    NG2 = NS // 2        # mlp groups of 2 samples

    nc = bacc.Bacc("TRN2", debug=False, num_devices=N_CORES)

    f3 = nc.dram_tensor("f3", [NS, N3, DIM], F32, kind="ExternalInput").ap()
    f4 = nc.dram_tensor("f4", [NS, N4, DIM], F32, kind="ExternalInput").ap()
    wq = nc.dram_tensor("wq", [DIM, DIM], BF16, kind="ExternalInput").ap()
    wk = nc.dram_tensor("wk", [DIM, DIM], BF16, kind="ExternalInput").ap()
    wv = nc.dram_tensor("wv", [DIM, DIM], BF16, kind="ExternalInput").ap()
    wr = nc.dram_tensor("wr", [DIM, DIM], FP8, kind="ExternalInput").ap()
    w1 = nc.dram_tensor("w1", [DIM, MLP_DIM], FP8, kind="ExternalInput").ap()
    w2 = nc.dram_tensor("w2", [MLP_DIM, DIM], FP8, kind="ExternalInput").ap()
    biasq = nc.dram_tensor("biasq", [N3, DIM], BF16, kind="ExternalInput").ap()
    biask_cm = nc.dram_tensor("biask_cm", [DIM, SG4 * N4], BF16, kind="ExternalInput").ap()
    br_row = nc.dram_tensor("br_row", [1, DIM], F32R, kind="ExternalInput").ap()
    b2_row = nc.dram_tensor("b2_row", [1, DIM], F32R, kind="ExternalInput").ap()
    bias1_cm = nc.dram_tensor("bias1_cm", [128, MLP_DIM // 128], F32, kind="ExternalInput").ap()
    ones_col = nc.dram_tensor("ones_col", [1, 128], F32R, kind="ExternalInput").ap()
    ident = nc.dram_tensor("ident", [128, 128], F32R, kind="ExternalInput").ap()
    out = nc.dram_tensor("out", [NS, N3, DIM], F32, kind="ExternalOutput").ap()

    with tile.TileContext(nc) as tc:
        # ---- pools alive for the whole kernel ----
        with (
            tc.tile_pool(name="consts", bufs=1) as cpool,
            tc.tile_pool(name="wattn", bufs=1) as wpool,
            tc.tile_pool(name="stats", bufs=1) as spool,
            tc.tile_pool(name="dram", bufs=1, space="DRAM") as dpool,
        ):
            ident_sb = cpool.tile([128, 128], F32R, tag="ident")
            nc.sync.dma_start(ident_sb[:], ident)
            identb_sb = cpool.tile([128, 128], BF16, tag="identb")
            nc.vector.tensor_copy(identb_sb[:], ident_sb[:])
            eps_sb = cpool.tile([128, 1], F32, tag="eps")
            nc.vector.memset(eps_sb[:], EPS)
            ones_sb = cpool.tile([1, 128], F32R, tag="ones")
            nc.sync.dma_start(ones_sb[:], ones_col)
            brrow_sb = cpool.tile([1, DIM], F32R, tag="brrow")
            b2row_sb = cpool.tile([1, DIM], F32R, tag="b2row")
            bq_sb = cpool.tile([128, 2, DIM], BF16, tag="bq")
            bk_sb = cpool.tile([128, 4, SG4 * N4], BF16, tag="bk")
            nc.sync.dma_start(bk_sb[:], biask_cm.rearrange("(c p) d -> p c d", p=128))
            b1_sb = cpool.tile([128, MLP_DIM // 128], F32, tag="b1")
            # persistent block-diagonal context tiles (off-diagonal stays zero)
            ctxbd = cpool.tile([128, 4, 128], BF16, tag="ctxbd")
            nc.vector.memset(ctxbd[:], 0.0)

            wq_sb = wpool.tile([128, 4, DIM], BF16, tag="wq")
            wk_sb = wpool.tile([128, 4, DIM], BF16, tag="wk")
            nc.sync.dma_start(wk_sb[:], wk.rearrange("(c p) d -> p c d", p=128))
            wv_sb = wpool.tile([128, 4, DIM], BF16, tag="wv")
            nc.sync.dma_start(wv_sb[:], wv.rearrange("(c p) d -> p c d", p=128))
            wr_sb = wpool.tile([128, 4, DIM], FP8, tag="wr")
            w1_sb = wpool.tile([128, 4, MLP_DIM], FP8, tag="w1")
            w2_sb = wpool.tile([128, 16, DIM], FP8, tag="w2")

            # f3 + attention output, resident in SBUF across both stages
            f3o_sb = spool.tile([128, 2 * NS, DIM], BF16, tag="f3o_all")
            stats3 = spool.tile([128, 2 * NS, 2], F32, tag="stats3")
            negm3 = spool.tile([128, 2 * NS], F32, tag="negm3")
            s3 = spool.tile([128, 2 * NS], F32, tag="s3")

            for _rep in range(repeat):
                # ================= STAGE 1 =================
                with (
                    tc.tile_pool(name="s1_sb", bufs=2) as p1,
                    tc.tile_pool(name="s1_sb3", bufs=3) as p13,
                    tc.tile_pool(name="kv", bufs=3) as pkv,
                    tc.tile_pool(name="ps_mm", bufs=1, space="PSUM") as ps_mm,
                    tc.tile_pool(name="ps_ctx", bufs=1, space="PSUM") as ps_ctx,
                    tc.tile_pool(name="ps_att", bufs=1, space="PSUM") as ps_att,
                ):
                    def inv_std_from_var(var_view, sinv_view, n_cols, tag):
                        """sinv = exp(-0.5*ln(var+eps)) on [128, n_cols] views."""
                        lnv = p1.tile([128, n_cols], F32, tag=f"lnv_{tag}")
                        nc.scalar.activation(lnv[:], var_view, ACTF.Ln, bias=eps_sb[:])
                        nc.scalar.activation(sinv_view, lnv[:], ACTF.Exp, scale=-0.5)

                    def f4_block(g):
                            # ---------- f4 block: SG4 samples ----------
                            ntb = SG4 // 2  # token-chunks of 128 (2 samples each)
                            x4 = p1.tile([128, ntb, DIM], F32, tag="x4")
                            for t in range(ntb):
                                nc.sync.dma_start(
                                    x4[:, t, :],
                                    f4[SG4 * g + 2 * t: SG4 * g + 2 * t + 2].rearrange(
                                        "a b d -> (a b) d"
                                    ),
                                )
                            mv4 = p1.tile([128, ntb, 2], F32, tag="mv4")
                            for t in range(ntb):
                                bns = p1.tile([128, 6], F32, tag="bns4")
                                nc.vector.bn_stats(bns[:], x4[:, t, :])
                                nc.vector.bn_aggr(mv4[:, t, :], bns[:])
                            sinv4 = p1.tile([128, ntb], F32, tag="sinv4")
                            inv_std_from_var(mv4[:, :, 1], sinv4[:], ntb, "s4")
                            negms4 = p1.tile([128, ntb], F32, tag="negms4")
                            nc.vector.scalar_tensor_tensor(
                                negms4[:], mv4[:, :, 0], -1.0, sinv4[:],
                                op0=ALU.mult, op1=ALU.mult,
                            )
                            x4h = p1.tile([128, ntb, DIM], BF16, tag="x4h")
                            for t in range(ntb):
                                nc.scalar.activation(
                                    x4h[:, t, :], x4[:, t, :], ACTF.Identity,
                                    bias=negms4[:, t: t + 1], scale=sinv4[:, t: t + 1],
                                )
                            # channel-major [512ch, SG4*64 tok] via DMA XBAR
                            x4c = p1.tile([128, 4, SG4 * N4], BF16, tag="x4c", bufs=2)
                            for t in range(ntb):
                                nc.sync.dma_start_transpose(
                                    x4c[:, :, t * 128:(t + 1) * 128], x4h[:, t, :]
                                )
                            # K projection (channel-major out) + bias preload + exp
                            kx = p1.tile([128, 4, SG4 * N4], BF16, tag="kx")
                            ks = p1.tile([128, 4 * SG4], F32, tag="ks")
                            for cc in range(4):
                                pk = ps_mm.tile([128, 512], F32, tag="mmk", bufs=1)
                                nc.tensor.matmul(
                                    pk[:], identb_sb[:], bk_sb[:, cc, :],
                                    start=True, stop=False,
                                )
                                for kc in range(4):
                                    nc.tensor.matmul(
                                        pk[:],
                                        wk_sb[:, kc, cc * 128:(cc + 1) * 128],
                                        x4c[:, kc, :],
                                        start=False, stop=(kc == 3),
                                    )
                                nc.scalar.activation(kx[:, cc, :], pk[:], ACTF.Exp)
                                nc.vector.reduce_sum(
                                    ks[:, cc * SG4:(cc + 1) * SG4],
                                    kx[:, cc, :].rearrange("p (s d) -> p s d", s=SG4),
                                    axis=AX,
                                )
                            kr = pkv.tile([128, 4 * SG4], F32, tag="kr")
                            nc.vector.reciprocal(kr[:], ks[:])
                            # V projection (token-major out) + bias preload
                            v_tm = pkv.tile([128, ntb, DIM], BF16, tag="v_tm")
                            for t in range(ntb):
                                pv = ps_mm.tile([128, 512], F32, tag="mmk", bufs=1)
                                for kc in range(4):
                                    nc.tensor.matmul(
                                        pv[:],
                                        x4c[:, kc, t * 128:(t + 1) * 128],
                                        wv_sb[:, kc, :],
                                        start=(kc == 0), stop=(kc == 3),
                                    )
                                nc.scalar.activation(v_tm[:, t, :], pv[:], ACTF.Copy)
                            # k back to token-major (bf16) via DMA XBAR transpose
                            k_tm = pkv.tile([128, ntb, DIM], BF16, tag="k_tm")
                            for cc in range(4):
                                nc.sync.dma_start_transpose(
                                    k_tm[:, :, cc * 128:(cc + 1) * 128], kx[:, cc, :]
                                )

                            return k_tm, v_tm, kr

                    def x3_load(s):
                        x3 = p13.tile([128, 2, DIM], F32, tag="x3", name="x3")
                        for t in range(2):
                            nc.sync.dma_start(
                                x3[:, t, :], f3[s, t * 128:(t + 1) * 128, :]
                            )
                        return x3

                    x3state = {}
                    x3state[0] = x3_load(0)
                    x3state[1] = x3_load(1)
                    kvstate = {}
                    kvstate[0] = f4_block(0)
                    # deferred loads: first needed ~8-12us in (Q/Wr of sample 0)
                    nc.sync.dma_start(bq_sb[:], biasq.rearrange("(t p) d -> p t d", p=128))
                    nc.sync.dma_start(wq_sb[:], wq.rearrange("(c p) d -> p c d", p=128))
                    nc.sync.dma_start(wr_sb[:], wr.rearrange("(c p) d -> p c d", p=128))
                    nc.sync.dma_start(brrow_sb[:], br_row)
                    if NG4 > 1:
                        kvstate[1] = f4_block(1)
                    # W1/W2 loads deferred here so startup DMA bandwidth goes
                    # to the first groups' activations and attention weights.
                    nc.sync.dma_start(w1_sb[:], w1.rearrange("(c p) d -> p c d", p=128))
                    nc.sync.dma_start(w2_sb[:], w2.rearrange("(c p) d -> p c d", p=128))
                    nc.sync.dma_start(b1_sb[:], bias1_cm)
                    nc.sync.dma_start(b2row_sb[:], b2_row)
                    for g in range(NG4):
                        k_tm, v_tm, kr = kvstate.pop(g)
                        # ---------- f3 blocks: SG4 samples ----------
                        for si in range(SG4):
                            if si == 4 and g + 2 < NG4:
                                kvstate[g + 2] = f4_block(g + 2)
                            s = SG4 * g + si
                            tb = si // 2
                            pb = (si % 2) * 64
                            x3 = x3state.pop(s)
                            if s + 2 < NS:
                                x3state[s + 2] = x3_load(s + 2)
                            mv1 = p1.tile([128, 2, 2], F32, tag="mv1")
                            for t in range(2):
                                bns1 = p1.tile([128, 6], F32, tag="bns1")
                                nc.vector.bn_stats(bns1[:], x3[:, t, :])
                                nc.vector.bn_aggr(mv1[:, t, :], bns1[:])
                            sinv1 = p1.tile([128, 2], F32, tag="sinv1")
                            inv_std_from_var(mv1[:, :, 1], sinv1[:], 2, "s1")
                            negms1 = p1.tile([128, 2], F32, tag="negms1")
                            nc.vector.scalar_tensor_tensor(
                                negms1[:], mv1[:, :, 0], -1.0, sinv1[:],
                                op0=ALU.mult, op1=ALU.mult,
                            )
                            x3h = p1.tile([128, 2, DIM], BF16, tag="x3h", bufs=2)
                            for t in range(2):
                                nc.scalar.activation(
                                    x3h[:, t, :], x3[:, t, :], ACTF.Identity,
                                    bias=negms1[:, t: t + 1], scale=sinv1[:, t: t + 1],
                                )
                            x3c = p1.tile([128, 4, 256], BF16, tag="x3c", bufs=2)
                            for t in range(2):
                                nc.sync.dma_start_transpose(
                                    x3c[:, :, t * 128:(t + 1) * 128], x3h[:, t, :]
                                )
                            # Q projection + biasq preload, then exp over both chunks
                            e_tm = p1.tile([128, 2, DIM], BF16, tag="e_tm", bufs=2)
                            for t in range(2):
                                pq = ps_mm.tile([128, 512], F32, tag="mmq", name="pq", bufs=3)
                                nc.tensor.matmul(
                                    pq[:], identb_sb[:], bq_sb[:, t, :],
                                    start=True, stop=False,
                                )
                                for kc in range(4):
                                    nc.tensor.matmul(
                                        pq[:],
                                        x3c[:, kc, t * 128:(t + 1) * 128],
                                        wq_sb[:, kc, :],
                                        start=False, stop=(kc == 3),
                                    )
                                nc.scalar.activation(e_tm[:, t, :], pq[:], ACTF.Exp)
                            qs = p1.tile([128, 16], F32, tag="qs")
                            nc.vector.reduce_sum(
                                qs[:],
                                e_tm.rearrange("p a (h d) -> p (a h) d", h=8)[:],
                                axis=AX,
                            )
                            qr = p1.tile([128, 16], BF16, tag="qr")
                            with nc.allow_low_precision(reason="softmax norm 1/sum in bf16"):
                                nc.vector.reciprocal(qr[:], qs[:])
                            q_tm = p1.tile([128, 2, DIM], BF16, tag="q_tm", bufs=2)
                            nc.vector.tensor_tensor(
                                q_tm.rearrange("p a (h d) -> p (a h) d", h=8)[:],
                                e_tm.rearrange("p a (h d) -> p (a h) d", h=8)[:],
                                qr[:].unsqueeze(-1).broadcast_to([128, 16, 64]),
                                op=ALU.mult,
                            )
                            q_cm = p1.tile([128, 4, 256], BF16, tag="q_cm", bufs=2)
                            for t in range(2):
                                nc.sync.dma_start_transpose(
                                    q_cm[:, :, t * 128:(t + 1) * 128], q_tm[:, t, :]
                                )
                            # attention per head-pair
                            att_cm = p1.tile([128, 4, 256], FP8, tag="att_cm")
                            for hp in range(4):
                                pctx = ps_ctx.tile([128, 128], F32, tag="ctx")
                                nc.tensor.matmul(
                                    pctx[:],
                                    k_tm[pb:pb + 64, tb, hp * 128:(hp + 1) * 128],
                                    v_tm[pb:pb + 64, tb, hp * 128:(hp + 1) * 128],
                                    start=True, stop=True,
                                )
                                for hh in range(2):
                                    nc.vector.tensor_scalar_mul(
                                        ctxbd[hh * 64:(hh + 1) * 64, hp, hh * 64:(hh + 1) * 64],
                                        pctx[hh * 64:(hh + 1) * 64, hh * 64:(hh + 1) * 64],
                                        kr[hh * 64:(hh + 1) * 64, hp * SG4 + si: hp * SG4 + si + 1],
                                    )
                                patt = ps_att.tile([128, 256], F32, tag="att")
                                nc.tensor.matmul(
                                    patt[:], ctxbd[:, hp, :], q_cm[:, hp, :],
                                    start=True, stop=True,
                                )
                                nc.scalar.activation(att_cm[:, hp, :], patt[:], ACTF.Copy)
                            # Wr + residual (PE ident10 accumulate) -> f3out
                            for t in range(2):
                                po = ps_mm.tile([128, 512], F32, tag="mmq", bufs=3)
                                nc.tensor.matmul(po[:], ones_sb[:], brrow_sb[:], start=True, stop=False)
                                for cc in range(2):
                                    nc.tensor.matmul(
                                        po[:],
                                        att_cm[:, 2 * cc:2 * cc + 2, t * 128:(t + 1) * 128],
                                        wr_sb[:, 2 * cc:2 * cc + 2, :],
                                        start=False, stop=(cc == 1),
                                        perf_mode=DR,
                                    )
                                f3o = f3o_sb[:, 2 * s + t, :]
                                nc.vector.scalar_tensor_tensor(
                                    f3o, po[:], DESCALE, x3[:, t, :],
                                    op0=ALU.mult, op1=ALU.add,
                                )
                                bns3 = p1.tile([128, 6], F32, tag="bns3")
                                nc.vector.bn_stats(bns3[:], f3o)
                                nc.vector.bn_aggr(stats3[:, 2 * s + t, :], bns3[:])

                    # LN3 stats math for all samples (natural_log_exp table
                    # is active here; stage 2 then avoids table reloads)
                    nc.vector.tensor_scalar_mul(negm3[:], stats3[:, :, 0], -1.0)
                    lnv3 = p1.tile([128, 2 * NS], F32, tag="lnv3")
                    nc.scalar.activation(lnv3[:], stats3[:, :, 1], ACTF.Ln, bias=eps_sb[:])
                    nc.scalar.activation(s3[:], lnv3[:], ACTF.Exp, scale=-0.5)

                # ================= STAGE 2 =================
                with (
                    tc.tile_pool(name="s2_sb", bufs=3) as p2,
                    tc.tile_pool(name="s2_sb3", bufs=3) as p23,
                    tc.tile_pool(name="ps2_w1", bufs=2, space="PSUM") as ps2_w1,
                    tc.tile_pool(name="ps2_w2", bufs=4, space="PSUM") as ps2_w2,
                ):
                    for g in range(NG2):
                        # LN3 apply on DVE (bf16 in/out, all-SBUF -> fast path)
                        xoh = p2.tile([128, 4, DIM], BF16, tag="xoh")
                        for c in range(4):
                            col = 4 * g + c
                            nc.vector.tensor_scalar(
                                xoh[:, c, :], f3o_sb[:, col, :],
                                negm3[:, col: col + 1], s3[:, col: col + 1],
                                op0=ALU.add, op1=ALU.mult,
                            )
                        xoc_bf = p2.tile([128, 4, DIM], BF16, tag="xoc_bf")
                        for c in range(4):
                            nc.sync.dma_start_transpose(
                                xoc_bf[:, :, c * 128:(c + 1) * 128], xoh[:, c, :]
                            )
                        xoc = p2.tile([128, 4, DIM], FP8, tag="xoc")
                        for h in range(2):
                            nc.vector.tensor_copy(
                                xoc[:, 2 * h:2 * h + 2, :], xoc_bf[:, 2 * h:2 * h + 2, :]
                            )
                        pf = [ps2_w2.tile([128, 512], F32, tag="w2acc", name="pfc")
                              for _ in range(4)]
                        for hp in range(8):
                            gt2 = p23.tile([128, 2, DIM], FP8, tag="gt")
                            for i in range(2):
                                hc = 2 * hp + i
                                pw1 = ps2_w1.tile([128, 512], F32, tag="w1ps")
                                for kc in range(2):
                                    nc.tensor.matmul(
                                        pw1[:],
                                        w1_sb[:, 2 * kc:2 * kc + 2, hc * 128:(hc + 1) * 128],
                                        xoc[:, 2 * kc:2 * kc + 2, :],
                                        start=(kc == 0), stop=(kc == 1),
                                        perf_mode=DR,
                                    )
                                nc.scalar.activation(
                                    gt2[:, i, :], pw1[:], ACTF.Gelu,
                                    bias=b1_sb[:, hc: hc + 1], scale=DESCALE,
                                )
                            for c in range(4):
                                nc.tensor.matmul(
                                    pf[c][:],
                                    gt2[:, :, c * 128:(c + 1) * 128],
                                    w2_sb[:, 2 * hp:2 * hp + 2, :],
                                    start=(hp == 0), stop=False,
                                    perf_mode=DR,
                                    skip_group_check=(hp > 0),
                                )
                        for c in range(4):
                            nc.tensor.matmul(
                                pf[c][:], ones_sb[:], b2row_sb[:],
                                start=False, stop=True,
                            )
                        for c in range(4):
                            outt = p2.tile([128, DIM], F32, tag="outt")
                            nc.vector.scalar_tensor_tensor(
                                outt[:], pf[c][:], DESCALE, f3o_sb[:, 4 * g + c, :],
                                op0=ALU.mult, op1=ALU.add,
                            )
                            nc.sync.dma_start(
                                out[2 * g + c // 2, (c % 2) * 128:(c % 2) * 128 + 128, :],
                                outt[:],
                            )

    nc.compile()
    return nc


def _get_module(n_samples):
    if n_samples not in _BUILD_CACHE:
        _BUILD_CACHE[n_samples] = _build(n_samples)
    return _BUILD_CACHE[n_samples]


def kernel(**inputs) -> np.ndarray:
    from concourse.bass_utils import run_bass_kernel_spmd

    consts = _host_prep(inputs)
    f3 = np.ascontiguousarray(np.asarray(inputs["f3"], dtype=np.float32))
    f4 = np.ascontiguousarray(np.asarray(inputs["f4"], dtype=np.float32))

    nc = _get_module(BSH)
    in_maps = []
    for c in range(N_CORES):
        m = dict(consts)
        m["f3"] = np.ascontiguousarray(f3[c * BSH:(c + 1) * BSH])
        m["f4"] = np.ascontiguousarray(f4[c * BSH:(c + 1) * BSH])
        in_maps.append(m)
    res = run_bass_kernel_spmd(nc, in_maps, core_ids=list(range(N_CORES)))
    return np.concatenate([res.results[c]["out"] for c in range(N_CORES)], axis=0)

